# revision 1
# baseline (speedup 1.0000x reference)
"""AttentiveMMDPrompt.compute_attn_weight kernel for 8 Trainium2 NeuronCores.

Strategy (data-parallel over episodes b=8, one episode per core):

  Per episode, the only heavy compute is the local projection
  l = local_f @ Wk^T  (14700x640 @ 640x640 = 12 GF fp32).  Everything the
  reference needs downstream of l can be expressed as columns of ONE widened
  matmul  Z = x @ R, where x = local_f tokens [14700, 640] and
  R = [Wk^T | Wk^T @ ghat_g (g=0..4) | Wk^T @ mean]  (640 x 646):

    Z[:, 0:640]   = l                      -> row sumsq gives |l_i|^2
    Z[:, 640+g]   = l_i . ghat_g           -> raw attention scores
    Z[:, 645]     = l_i . mean             -> centering correction

  |l_i - mean|^2 = |l_i|^2 - 2 l_i.mean + |mean|^2, so the l2-normalized,
  centered cosine scores come out of per-token scalars only - the big l
  tensor never leaves PSUM.  The g-side (5 vectors), the episode mean and R
  are tiny and are precomputed on host in float64.

  The big matmul runs in float32r (FP22 multiply, FP32 accumulate,
  1 cycle/row, 4x faster than true fp32; end-to-end attn error ~3e-6 absmax,
  validated against the fp64 reference).

  Softmax groups (196 tokens per (l, g)) cross the 128-partition token
  chunks, so group sums / broadcasts go through tiny PE matmuls against a
  host-provided 0/1 group-indicator matrix B [14720, 75] and its transpose.

  Device inputs per core (host pre-transposed / padded, a pure layout step):
    xT     [640, 14720] f32  - local tokens, c-major, padded 14700->14720
    rmat   [5, 128, 648] f32 - R in 5 contraction chunks, z-padded to 648
    consts [6]           f32 - [mean.ghat_g (5) | alpha^2*(|mean|^2+eps)]
    bmat   [128, 115, 75] f32 - B chunks, partition-major
    btmat  [75, 14720]    f32 - B^T
  Output per core:
    O [5, 14720] f32 - normalized attention, token-major per g; host slices
    the 20 pad tokens and reshapes to [75, 5, 196, 1].
"""

import numpy as np

import bass_rust
import concourse.bass as bass
import concourse.mybir as mybir
import concourse.tile as tile
from concourse.bass_utils import run_bass_kernel_spmd
from concourse.masks import make_identity

# Problem shapes (hardcoded per contract).
B, NG, NL, NF, C = 8, 5, 75, 196, 640
ALPHA, EPS = 0.1, 1e-12
NT = NL * NF            # 14700 tokens per episode
TCH = 115               # token chunks of 128
NTP = TCH * 128         # 14720 (padded)
CCH = 5                 # contraction chunks of 128 (C = 640)
ZP = 648                # z columns: 640 proj | 5 scores | 1 l.mean | 2 pad
ZA, ZB = 320, 328       # z split (two PSUM banks; both >=256 for f32r rate)
# Epilogue phase boundaries (chunk indices).  Short final phase keeps the
# serial kernel tail (last stats + exp + group sums) small.
PHASE_ENDS = [29, 58, 87, 109, TCH]
F32 = mybir.dt.float32
F32R = mybir.dt.float32r
AF = mybir.ActivationFunctionType
ALU = mybir.AluOpType


def _split_multi_waits(nc: bass.Bass) -> None:
    """Rewrite the BIR so no instruction carries more than one sem wait.

    The walrus build in this container rejects instructions with more than
    one sync-wait command (CoreV3 setupSyncWait, all encodings).  Extra
    waits are hoisted onto no-op instructions inserted immediately before
    the owner on the same engine: waits execute in program order per
    engine sequencer, so satisfying them one instruction earlier on the
    same engine is semantically identical.
    """
    for f in nc.m.functions:
        for b in f.blocks:
            insts = list(b.instructions)
            out = []
            changed = False
            for inst in insts:
                si = inst.sync_info
                if si is not None and len(si.on_wait) > 1:
                    waits = list(si.on_wait)
                    for w in waits[:-1]:
                        nop = mybir.InstNoOp(
                            name=nc.get_next_instruction_name(), ins=[], outs=[]
                        )
                        nop.engine = inst.engine
                        nop.sync_info = bass_rust.SyncInfo(
                            on_wait=[w], on_update=[]
                        )
                        nc.register_instruction(nop)
                        out.append(nop)
                    inst.sync_info = bass_rust.SyncInfo(
                        on_wait=[waits[-1]], on_update=list(si.on_update)
                    )
                    changed = True
                out.append(inst)
            if changed:
                b.instructions = out


def _build_program(n_rep: int = 1, mm_dtype=F32R) -> bass.Bass:
    """Build the per-core program.  n_rep>1 repeats the whole computation
    (timing harness only - isolates device time from dispatch overhead).
    mm_dtype switches the main-matmul operand dtype (bf16 = timing probe)."""
    nc = bass.Bass(
        "TRN2",
        target_bir_lowering=False,
        debug=False,
        enable_asserts=True,
        num_devices=B,
    )
    xT = nc.dram_tensor("xT", [C, NTP], mm_dtype, kind="ExternalInput")
    rmat = nc.dram_tensor("rmat", [CCH, 128, ZP], mm_dtype, kind="ExternalInput")
    consts = nc.dram_tensor("consts", [NG + 1], F32, kind="ExternalInput")
    bmat = nc.dram_tensor("bmat", [128, TCH, NL], mybir.dt.uint8,
                          kind="ExternalInput")
    btmat = nc.dram_tensor("btmat", [NL, NTP], mybir.dt.uint8,
                           kind="ExternalInput")
    O = nc.dram_tensor("O", [NG, NTP], F32, kind="ExternalOutput")

    with tile.TileContext(nc, num_cores=B) as tc:
        with (
            tc.tile_pool(name="singles", bufs=1) as singles,
            tc.tile_pool(name="xin", bufs=4) as xin,
            tc.tile_pool(name="sq", bufs=2) as sqp,
            tc.tile_pool(name="zpsum", bufs=2, space="PSUM") as zpsum,
            tc.tile_pool(name="gpsum", bufs=1, space="PSUM") as gpsum,
            tc.tile_pool(name="tpsum", bufs=2, space="PSUM") as tpsum,
        ):
            # ---- one-time loads -------------------------------------------------
            # rm split per contraction chunk so the first matmuls can start
            # as soon as their slice lands.
            rm = singles.tile([128, CCH, ZP], mm_dtype)
            rmr = rmat.rearrange("cc p z -> p cc z")
            nc.sync.dma_start(out=rm[:, 0, :], in_=rmr[:, 0, :])

            # bsb/btsb are loaded lazily inside the first episode (after the
            # first phase of x chunks) to keep startup DMA off the critical
            # path; they arrive as uint8 and are widened to f32 on DVE so the
            # DMA pipe only carries 1/4 of the bytes.
            bsb_u8 = singles.tile([128, TCH, NL], mybir.dt.uint8)
            btsb_u8 = singles.tile([NL, TCH, 128], mybir.dt.uint8)
            bsb = singles.tile([128, TCH, NL], F32)
            btsb = singles.tile([NL, TCH, 128], F32)

            cg = singles.tile([128, NG], F32)
            nc.sync.dma_start(out=cg, in_=consts[0:NG].partition_broadcast(128))
            m2e = singles.tile([128, 1], F32)
            nc.sync.dma_start(
                out=m2e, in_=consts[NG : NG + 1].partition_broadcast(128)
            )

            ident = singles.tile([128, 128], F32)
            make_identity(nc, ident)

            # ---- persistent per-token stats -------------------------------------
            ssa = singles.tile([128, TCH], F32)     # |l|^2 per token
            sclm = singles.tile([128, 6, TCH], F32)  # 5 raw scores + l.mean
            tmp0 = singles.tile([128, TCH], F32)
            nrm = singles.tile([128, TCH], F32)
            inv = singles.tile([128, TCH], F32)
            sfin = singles.tile([128, NG, TCH], F32)
            ebuf = singles.tile([128, NG, TCH], F32)
            abuf = singles.tile([128, NG, TCH], F32)
            obuf = singles.tile([TCH, NG, 128], F32)
            rgs = singles.tile([NL, NG], F32)

            xTr = xT.rearrange("(cc p) i -> p cc i", p=128)

            for _rep in range(n_rep):
                first = _rep == 0
                _emit_episode(nc, locals())

    _split_multi_waits(nc)
    return nc


def _emit_episode(nc, env):
    (xin, zpsum, sqp, gpsum, tpsum) = (
        env["xin"], env["zpsum"], env["sqp"], env["gpsum"], env["tpsum"],
    )
    (rm, bsb, btsb, cg, m2e, ident) = (
        env["rm"], env["bsb"], env["btsb"], env["cg"], env["m2e"], env["ident"],
    )
    (ssa, sclm, tmp0, nrm, inv, sfin, ebuf, abuf, obuf, rgs) = (
        env["ssa"], env["sclm"], env["tmp0"], env["nrm"],
        env["inv"], env["sfin"], env["ebuf"], env["abuf"], env["obuf"],
        env["rgs"],
    )
    xTr, O = env["xTr"], env["O"]
    if True:
        if True:
            # ---- main pass: Z = x @ R, consume Z in PSUM, phased epilogue -------
            # Phases let the softmax stats/exp/group-sum work for early chunks
            # overlap the later chunks' matmuls instead of serializing at the
            # end of the kernel.
            gs = gpsum.tile([NL, NG], F32)
            phase_ends = PHASE_ENDS
            ph_start = 0
            for ph_end in phase_ends:
                for t in range(ph_start, ph_end):
                    xt = xin.tile([128, CCH, 128], env["mm_dtype"], tag="xt")
                    nc.sync.dma_start(
                        out=xt, in_=xTr[:, :, 128 * t : 128 * (t + 1)]
                    )
                    if t == 0 and ph_start == 0 and env["first"]:
                        for _cc in range(1, CCH):
                            nc.sync.dma_start(
                                out=rm[:, _cc, :], in_=env["rmr"][:, _cc, :]
                            )
                    # One 2-bank PSUM tile; z-half A at col 0 (bank 0), half B
                    # at col 512 (bank 1) so each matmul stays in one bank but
                    # ONE strided ACT op can square+accumulate both halves.
                    pz = zpsum.tile([128, 1024], F32, tag="pz")
                    for cc in range(CCH):
                        lhsT = xt[:, cc, :]
                        nc.tensor.matmul(
                            pz[:, 0:ZA],
                            lhsT,
                            rm[:, cc, 0:ZA],
                            start=(cc == 0),
                            stop=(cc == CCH - 1),
                        )
                        nc.tensor.matmul(
                            pz[:, 512 : 512 + ZB],
                            lhsT,
                            rm[:, cc, ZA:ZP],
                            start=(cc == 0),
                            stop=(cc == CCH - 1),
                        )
                    # |l|^2 = sum of squares over cols {0:320, 512:832}.
                    sq = sqp.tile([128, 2, ZA], F32, tag="sq")
                    nc.scalar.activation(
                        sq,
                        pz.rearrange("p (h c) -> p h c", h=2)[:, :, 0:ZA],
                        AF.Square,
                        accum_out=ssa[:, t : t + 1],
                    )
                    # raw scores (5) + l.mean (1).
                    nc.vector.tensor_copy(
                        sclm[:, :, t], pz[:, 512 + ZA : 512 + ZA + 6]
                    )

                # ---- per-phase normalization + exp + group-sum matmuls ----------
                sl = slice(ph_start, ph_end)
                # (l.mean * -2) + |l|^2
                nc.vector.scalar_tensor_tensor(
                    out=nrm[:, sl],
                    in0=sclm[:, 5, sl],
                    scalar=-2.0,
                    in1=ssa[:, sl],
                    op0=ALU.mult,
                    op1=ALU.add,
                )
                # alpha * |l - mean| (+eps): sqrt(a^2 x + a^2(|mean|^2 + eps))
                nc.scalar.activation(
                    tmp0[:, sl],
                    nrm[:, sl],
                    AF.Sqrt,
                    bias=m2e[:, 0:1],
                    scale=float(ALPHA * ALPHA),
                )
                nc.vector.reciprocal(inv[:, sl], tmp0[:, sl])
                # s = (raw - mean.ghat_g) * inv ; then E = exp(s)
                for g in range(NG):
                    nc.vector.scalar_tensor_tensor(
                        out=sfin[:, g, sl],
                        in0=sclm[:, g, sl],
                        scalar=cg[:, g : g + 1],
                        in1=inv[:, sl],
                        op0=ALU.subtract,
                        op1=ALU.mult,
                    )
                nc.scalar.activation(ebuf[:, :, sl], sfin[:, :, sl], AF.Exp)
                if ph_start == 0 and env["first"]:
                    # B (uint8, cast to f32 in the SWDGE DMA) is not needed
                    # until the first phase's group-sum matmuls.  A pure
                    # priority hint cannot stop the scheduler from issuing
                    # this dependency-free DMA at t=0, where it head-blocks
                    # the startup x-chunk loads for ~12us - so seed the tile
                    # with a copy of phase-0 data first: the WAW dependency
                    # forces the big load to wait until phase 0 is underway.
                    nc.vector.tensor_copy(env["bsb_u8"][0:1, 0, 0:1], ssa[0:1, 10:11])
                    nc.sync.dma_start(out=env["bsb_u8"], in_=env["bmat"][:, :, :])
                    nc.vector.tensor_copy(bsb, env["bsb_u8"])
                if ph_start == PHASE_ENDS[1] and env["first"]:
                    # B^T is only consumed by the final broadcast matmuls;
                    # same WAW-seeding trick, keyed on phase-2 data.
                    nc.vector.tensor_copy(
                        env["btsb_u8"][0:1, 0, 0:1],
                        ssa[0:1, PHASE_ENDS[1] - 1 : PHASE_ENDS[1]],
                    )
                    nc.sync.dma_start(
                        out=env["btsb_u8"],
                        in_=env["btmat"].rearrange("l (t p) -> l t p", p=128),
                    )
                    nc.vector.tensor_copy(btsb, env["btsb_u8"])
                for t in range(ph_start, ph_end):
                    nc.tensor.matmul(
                        gs[:, :],
                        bsb[:, t, :],
                        ebuf[:, :, t],
                        start=(t == 0),
                        stop=(t == TCH - 1),
                    )
                ph_start = ph_end

            nc.vector.reciprocal(rgs, gs[:, :])

            # ---- broadcast 1/sum back to tokens and normalize -------------------
            for t in range(TCH):
                r2 = tpsum.tile([128, NG], F32, tag="tail")
                nc.tensor.matmul(
                    r2[:, :], btsb[:, t, :], rgs[:, :], start=True, stop=True
                )
                nc.vector.tensor_mul(abuf[:, :, t], ebuf[:, :, t], r2[:, :])

            # ---- transpose to token-major and store -----------------------------
            for g in range(NG):
                tp = tpsum.tile([TCH, 128], F32, tag="tail")
                nc.tensor.transpose(tp[:, :], abuf[:, g, :], ident[:, :])
                nc.scalar.copy(obuf[:, g, :], tp[:, :])
            nc.sync.dma_start(
                out=O.rearrange("g (t p) -> t g p", p=128), in_=obuf
            )


_PROGRAM_CACHE: list = []


def _host_prep(global_f, local_f, Wq, Wk):
    """Per-episode host-side constant prep + layout marshaling -> in_maps."""
    gf = np.asarray(global_f, dtype=np.float32)
    lf = np.asarray(local_f, dtype=np.float32)
    Wq64 = np.asarray(Wq, dtype=np.float64)
    Wk64 = np.asarray(Wk, dtype=np.float64)

    # Episode-independent device tensors.
    tok = np.arange(NTP)
    grp = tok // NF
    bmat_full = ((grp[:, None] == np.arange(NL)[None, :]) & (tok[:, None] < NT))
    bmat_full = bmat_full.astype(np.uint8)                  # [14720, 75]
    bmat = np.ascontiguousarray(
        bmat_full.reshape(TCH, 128, NL).transpose(1, 0, 2)
    )                                                       # [128, 115, 75]
    btmat = np.ascontiguousarray(bmat_full.T)               # [75, 14720]

    in_maps = []
    for bi in range(B):
        x64 = lf[bi].reshape(NT, C).astype(np.float64)
        q = gf[bi].astype(np.float64) @ Wq64.T              # [5, 640]
        mean = (q.sum(0) + x64.sum(0) @ Wk64.T) / (NG + NT)
        gc_ = q - mean
        ghat = gc_ / np.sqrt((gc_ * gc_).sum(-1, keepdims=True) + EPS)
        R = np.zeros((C, ZP), np.float64)
        R[:, 0:C] = Wk64.T
        R[:, C : C + NG] = (ghat @ Wk64).T
        R[:, C + NG] = Wk64.T @ mean
        rmat = np.ascontiguousarray(
            R.astype(np.float32).reshape(CCH, 128, ZP)
        )
        consts = np.empty(NG + 1, np.float32)
        consts[0:NG] = ghat @ mean
        consts[NG] = (ALPHA * ALPHA) * (mean @ mean + EPS)

        xT = np.zeros((C, NTP), np.float32)
        xT[:, :NT] = lf[bi].reshape(NT, C).T
        in_maps.append(
            {
                "xT": np.ascontiguousarray(xT),
                "rmat": rmat,
                "consts": consts,
                "bmat": bmat,
                "btmat": btmat,
            }
        )
    return in_maps


def kernel(global_f, local_f, Wq, Wk):
    in_maps = _host_prep(global_f, local_f, Wq, Wk)

    if not _PROGRAM_CACHE:
        _PROGRAM_CACHE.append(_build_program())
    nc = _PROGRAM_CACHE[0]

    res = run_bass_kernel_spmd(nc, in_maps, core_ids=list(range(B)))

    out = np.empty((B, NL, NG, NF, 1), np.float32)
    for bi in range(B):
        Ob = res.results[bi]["O"][:, :NT]                   # [5, 14700]
        out[bi] = Ob.reshape(NG, NL, NF).transpose(1, 0, 2)[..., None]
    return out

